# revision 10
# baseline (speedup 1.0000x reference)
"""Trainium2 Bass kernel for nn_LilletLayer (gnn_message_passing) — v7.

Math per head h, molecule b (reference-exact algebra):
  att[a,c,n] = D2[a,c] * g[a,n] * g[c,n] over 15 canonical pairs, folded
  to 120 triangular pair-pairs (6000 rows); h1_h = W1_h^T att_h.

Split:
 - HOST (O(B*15) trivial prep, like the existing xc fold): coarse coords
   xc, pair deltas, distances, cutoff, 1/(d+1e-6)^2, D2 gram — shipped as
   e[b,a]=exp(-d) and d2fm[b,a,c]=D2*cutoff_a*cutoff_c/(d_a d_c)^2 (bf16).
 - DEVICE (one core per head): the O(B*6000) smearing g = exp(-beta*
   (e-mu)^2) (ACT Square+Exp — a single activation-table set, warmed at
   t=0), the 1.5M-element att outer products (DVE), 47 PE transposes and
   47 [128x128x128] bf16 matmuls accumulating h1_h in PSUM fp32.
 - HOST: sum the 8 h1 partials + silu + W2 + b2 (49 kFLOP). No device
   collective — an 8-rank AllReduce of 32KB costs ~50us wall here (mesh
   latency + launch skew) and is intermittently flaky.

Scheduling: g in two a-blocks (high first), the per-a att loop runs
descending a, transposes/copies/matmuls and the W1 DMA follow the same
descending order; one long PE warm burst promotes the PE clock before
the real transposes; PSUM->SBUF copies split ACT/DVE.
"""

import numpy as np

import concourse.bacc as bacc
import concourse.bass as bass
import concourse.mybir as mybir
import concourse.tile as tile
from concourse.bass_utils import run_bass_kernel_spmd
from concourse.masks import make_identity

B, N, H, K, R = 128, 512, 8, 6, 50
CUT = 5.0
P15 = K * (K - 1) // 2
NPAIR = P15 * (P15 + 1) // 2
FTOT = NPAIR * R              # 6000
NCH = 47
FPAD = NCH * 128              # 6016
HID = 128
XPK = P15 + R + 1             # packed fp32 input: e(15) means(50) nbs(1)
F32 = mybir.dt.float32
BF16 = mybir.dt.bfloat16
AF = mybir.ActivationFunctionType
ALU = mybir.AluOpType

GBLKS = ((10, 15), (5, 10), (0, 5))  # g blocks, high a first
GRP = 6       # transpose chunks per PSUM group
NWARM = 48
NDVE_COPY = 2  # trailing copy groups on DVE


def _bcast(ap, axis, count):
    dims = [list(d) for d in ap.ap]
    dims.insert(axis + 1, [0, count])
    return bass.AP(tensor=ap.tensor, offset=ap.offset, ap=dims)


def _with_dims(ap, dims):
    return bass.AP(
        tensor=ap.tensor, offset=ap.offset, ap=[list(ap.ap[0])] + [list(d) for d in dims]
    )


def build_program(n_cores=8):
    nc = bacc.Bacc(
        "TRN2",
        target_bir_lowering=False,
        debug=False,
        enable_asserts=False,
        num_devices=n_cores,
    )

    xpin = nc.dram_tensor("xpin", [B, XPK], F32, kind="ExternalInput").ap()
    dfmin = nc.dram_tensor("dfmin", [B, FTOT], BF16, kind="ExternalInput").ap()
    w1s = nc.dram_tensor("w1s", [128, NCH * HID], BF16, kind="ExternalInput").ap()
    h1outd = nc.dram_tensor("h1out", [HID, B], F32, kind="ExternalOutput").ap()

    # block a>=10 covers cols [5250,6016) -> full chunks 42..46;
    # block a>=5 covers cols >=3250 -> chunks 26..41; a<5 -> chunks 0..25.
    groups = ([list(range(42, 47))]
              + [list(range(26, 32)), list(range(32, 38)), list(range(38, 42))]
              + [list(range(i, min(i + GRP, 26))) for i in range(0, 26, GRP)])

    with tile.TileContext(nc) as tc:
        with (
            tc.tile_pool(name="singles", bufs=1) as singles,
            tc.tile_pool(name="g2v", bufs=2) as g2v,
            tc.tile_pool(name="ps_t", bufs=4, space="PSUM") as ps_t_pool,
            tc.tile_pool(name="ps_acc", bufs=1, space="PSUM") as ps_acc_pool,
            tc.tile_pool(name="ps_w", bufs=1, space="PSUM") as ps_w_pool,
        ):
            # ---------------- t=0: DMAs, table warm, identity ----------------
            ident = singles.tile([128, 128], BF16)
            make_identity(nc, ident)

            c_zero = singles.tile([128, 1], F32)
            nc.vector.memset(c_zero, 0.0)
            warm_ex = singles.tile([128, 1], F32)
            nc.scalar.activation(warm_ex, c_zero, AF.Exp)

            xp_sb = singles.tile([128, XPK], F32)
            nc.sync.dma_start(out=xp_sb, in_=xpin)
            e_sb = xp_sb[:, 0:P15]
            mrep_sb = xp_sb[:, P15:P15 + R]
            nbs_sb = xp_sb[:, XPK - 1:XPK]
            d2fmx_sb = singles.tile([128, FTOT], BF16)
            nc.sync.dma_start(out=d2fmx_sb[:, 5250:], in_=dfmin[:, 5250:])
            nc.sync.dma_start(out=d2fmx_sb[:, 3250:5250], in_=dfmin[:, 3250:5250])
            nc.sync.dma_start(out=d2fmx_sb[:, :3250], in_=dfmin[:, :3250])

            # W1 DMA in 4 slices, descending chunk order
            w1sb = singles.tile([128, NCH, HID], BF16)
            for lo, hi in ((42, 47), (26, 42), (0, 13), (13, 26)):
                nc.sync.dma_start(
                    out=w1sb[:, lo:hi],
                    in_=w1s[:, lo * HID:hi * HID],
                )

            # PE warm-up: one long sustained burst to promote the clock
            ps_warm = ps_w_pool.tile([128, 128], BF16, tag="warm")
            for _ in range(NWARM):
                nc.tensor.transpose(ps_warm, ident, ident)

            # ------------- smearing g (two a-blocks, high block first) -------------
            t_sb = singles.tile([128, P15, R], F32)
            tsq_sb = singles.tile([128, P15, R], F32)
            g_sb = singles.tile([128, P15, R], BF16)
            for lo, hi in GBLKS:
                nn_ = hi - lo
                nc.vector.tensor_sub(
                    t_sb[:, lo:hi],
                    _bcast(e_sb[:, lo:hi], 1, R),
                    _bcast(mrep_sb, 0, nn_),
                )
                nc.scalar.activation(tsq_sb[:, lo:hi], t_sb[:, lo:hi], AF.Square)
                nc.scalar.activation(
                    g_sb[:, lo:hi], tsq_sb[:, lo:hi], AF.Exp, scale=nbs_sb
                )

            # ---------------- att (dense 6016 cols, descending a) ----------------
            attb = singles.tile([128, FPAD], BF16)
            nc.gpsimd.memset(attb[:, FTOT:], 0.0)
            offs = []
            off = 0
            for a in range(P15):
                offs.append(off)
                off += (P15 - a) * R
            for a in [a for lo, hi in GBLKS for a in range(lo, hi)]:
                cc = P15 - a
                g2_t = g2v.tile([128, cc, R], BF16, tag="g2")
                nc.vector.tensor_mul(
                    g2_t,
                    _with_dims(g_sb[:, a], [[0, cc], [1, R]]),
                    _with_dims(g_sb[:, a], [[R, cc], [1, R]]),
                )
                # flat bf16 mul (2x DVE mode): att = g2 * d2fmx, all step-1
                nc.vector.tensor_mul(
                    _with_dims(attb[:, offs[a]:], [[1, cc * R]]),
                    _with_dims(g2_t[:], [[1, cc * R]]),
                    _with_dims(d2fmx_sb[:, offs[a]:], [[1, cc * R]]),
                )

            # ---------------- transpose + matmul (descending chunks) ----------------
            attTb = singles.tile([128, NCH, 128], BF16)
            ps_acc = ps_acc_pool.tile([HID, B], F32)
            mm = 0
            prev = None

            def mm_group(chunk_list):
                nonlocal mm
                for c in chunk_list:
                    nc.tensor.matmul(
                        ps_acc,
                        lhsT=w1sb[:, c],
                        rhs=attTb[:, c],
                        start=(mm == 0),
                        stop=(mm == NCH - 1),
                    )
                    mm += 1

            for gi, grp in enumerate(groups):
                pst = ps_t_pool.tile([128, GRP, 128], BF16, tag="pst")
                c_lo = grp[0]
                for c in grp:
                    nc.tensor.transpose(
                        pst[:, c - c_lo], attb[:, c * 128:(c + 1) * 128], ident
                    )
                n_in = len(grp)
                tail4 = gi >= len(groups) - 4
                if tail4 and (len(groups) - 1 - gi) % 2 == 0:
                    nc.vector.tensor_copy(attTb[:, c_lo:c_lo + n_in], pst[:, :n_in])
                else:
                    nc.scalar.copy(attTb[:, c_lo:c_lo + n_in], pst[:, :n_in])
                if prev is not None:
                    mm_group(prev)
                prev = grp
            mm_group(prev)
            assert mm == NCH

            # ---------------- partial h1 out ----------------
            h1_sb = singles.tile([HID, B], F32)
            nc.vector.tensor_copy(h1_sb, ps_acc)
            nc.sync.dma_start(out=h1outd, in_=h1_sb)

    nc.compile()
    return nc


def host_prep(x, W_map, means, betas, W1, b1, W2, b2):
    import ml_dtypes

    x = np.ascontiguousarray(np.asarray(x, np.float32))
    W_map = np.asarray(W_map, np.float32)
    means = np.asarray(means, np.float32)
    betas = np.asarray(betas, np.float32)
    W1 = np.asarray(W1, np.float32)

    # host prep (O(B*15) per head): coarse coords, pair geometry, cutoff
    xc_h = np.einsum('hkn,bnd->hbkd', W_map, x).astype(np.float64)  # (H,B,K,3)
    canon = [(i, j) for i in range(K) for j in range(i + 1, K)]
    ii = np.array([i for i, _ in canon])
    jj = np.array([j for _, j in canon])
    delta = xc_h[:, :, ii, :] - xc_h[:, :, jj, :]          # (H,B,15,3)
    d2 = (delta ** 2).sum(-1)
    dn = np.sqrt(d2)                                        # (H,B,15)
    e_h = np.exp(-dn)
    cutoff = 0.5 * (np.cos(dn * np.pi / CUT) + 1.0) * (dn < CUT)
    m3 = cutoff / (dn + 1e-6) ** 2                          # (H,B,15)
    d2f = np.einsum('hbad,hbcd->hbac', delta, delta)        # (H,B,15,15)
    d2fm = d2f * m3[:, :, :, None] * m3[:, :, None, :]      # (H,B,15,15)
    # expand over n into the dense (a, c>=a, n) layout matching att
    d2fmx = np.empty((H, B, FTOT), np.float32)
    off = 0
    for a in range(P15):
        cc = P15 - a
        d2fmx[:, :, off:off + cc * R] = np.repeat(
            d2fm[:, :, a, a:], R, axis=-1
        ).reshape(H, B, cc * R)
        off += cc * R
    d2fm_dev = np.ascontiguousarray(d2fmx.astype(ml_dtypes.bfloat16))

    # fold W1 onto the 120 triangular canonical pair-pairs
    P36 = K * K
    a_of = np.array([i * K + j for (i, j) in canon])
    abar = np.array([j * K + i for (i, j) in canon])
    W1r = W1.reshape(H, P36, P36, R, HID)
    W1q = (
        W1r[:, a_of[:, None], a_of[None, :]]
        - W1r[:, a_of[:, None], abar[None, :]]
        - W1r[:, abar[:, None], a_of[None, :]]
        + W1r[:, abar[:, None], abar[None, :]]
    )
    tri_a, tri_c = np.triu_indices(P15)
    W1t = W1q[:, tri_a, tri_c] + np.where(
        (tri_a != tri_c)[None, :, None, None], W1q[:, tri_c, tri_a], 0.0
    )
    W1flat = np.zeros((H, FPAD, HID), np.float32)
    W1flat[:, :FTOT] = W1t.reshape(H, FTOT, HID)
    W1s_dev = np.ascontiguousarray(
        W1flat.reshape(H, NCH, 128, HID).transpose(0, 2, 1, 3).reshape(H, 128, NCH * HID)
        .astype(ml_dtypes.bfloat16)
    )

    assert np.all(betas == betas[0]), "kernel folds the uniform beta into Exp"
    xpack = np.zeros((H, B, XPK), np.float32)
    xpack[:, :, :P15] = e_h
    xpack[:, :, P15:P15 + R] = means[None, None, :]
    xpack[:, :, XPK - 1] = -float(betas[0])

    return [
        dict(
            xpin=np.ascontiguousarray(xpack[h]),
            dfmin=d2fm_dev[h],
            w1s=W1s_dev[h],
        )
        for h in range(H)
    ]


_NC_CACHE = {}


def get_program():
    if "nc" not in _NC_CACHE:
        _NC_CACHE["nc"] = build_program()
    return _NC_CACHE["nc"]


def kernel(x, W_map, means, betas, W1, b1, W2, b2, _debug=False, _trace=False):
    in_maps = host_prep(x, W_map, means, betas, W1, b1, W2, b2)
    nc = get_program()
    res = run_bass_kernel_spmd(nc, in_maps, list(range(H)), trace=_trace)
    h1 = np.zeros((HID, B), np.float64)
    for r in res.results:
        h1 += np.asarray(r["h1out"], np.float32)
    b1 = np.asarray(b1, np.float64).reshape(HID, 1)
    W2v = np.asarray(W2, np.float64).reshape(HID)
    z = h1 + b1
    sig = 1.0 / (1.0 + np.exp(-z))
    out = (W2v @ (z * sig)) + float(np.asarray(b2).reshape(()))
    if _debug or _trace:
        kernel.last_results = res
    return out[:, None].astype(np.float32)
